# revision 45
# baseline (speedup 1.0000x reference)
"""Trainium2 Bass kernel for nn_ActorMultiEnv_v2 (NALU actor network).

Strategy:
  - Data-parallel: 16384 tokens sharded as 2048 tokens per core across 8 cores.
  - Feature-major on-chip layout: activations stored [feature(partition), token(free)],
    so every matmul uses the weight in natural [in, out] layout as the stationary
    operand and never needs an on-device transpose.
  - All matmuls run as float32r (full fp32 data, reduced-precision multiplier,
    1 cycle/row on the PE -- 4x faster than true fp32).
  - sigmoid(g) is computed as 0.5 + 0.5*tanh(g/2) so the whole network needs only
    two ACT table sets (natural_log_exp and gelu_apprx_tanh); token superchunks are
    staggered through the two table "eras" to keep table swaps ~O(10).
  - Host does weight-only preparation (tanh(W_hat)*sigmoid(M_hat), the block-diagonal
    Keff = kron(I, kn)[:f1,:f1]*we, W3*we, bias folds) and input/output layout
    (transpose to feature-major shards, gather + transpose back).
"""

import sys

import numpy as np

for _p in ("/opt/trn_rl_repo",):
    if _p not in sys.path:
        sys.path.append(_p)

# ---- problem constants (hardcoded per harness contract) ----
B, T, FEAT, F1 = 128, 128, 256, 512
ACTION_DIM, ATOMS = 6, 32
SZ = ACTION_DIM * ATOMS            # 192
NCORES = 8
NTOK = B * T                        # 16384
TOK = NTOK // NCORES                # 2048 tokens per core
SUP = 4                             # superchunks per core
TS = TOK // SUP                     # 512 tokens per superchunk (== one PSUM chunk)
P = 128
M1 = F1 // P                        # 4 output-feature tiles
K1 = FEAT // P                      # 2 input k-tiles (layer 1)
K2 = F1 // P                        # 4 k-tiles (f1-sized contractions)
KE = 2 * F1 // P                    # 8 k-tiles (fc1e)
MO = 2                              # o-matmul M tiles (128 + 64)

EXP_LO = float(np.exp(np.float64(np.float32(-20.0))))
EXP_HI = float(np.exp(np.float64(np.float32(20.0))))

_CACHE = {}


def _build_program():
    from contextlib import ExitStack

    import concourse.tile as tile
    from concourse import bacc, mybir

    f32 = mybir.dt.float32
    f32r = mybir.dt.float32r
    AFT = mybir.ActivationFunctionType
    ALU = mybir.AluOpType

    nc = bacc.Bacc("TRN2", target_bir_lowering=False, debug=False,
                   num_devices=NCORES)

    def din(name, shape, dt=f32):
        return nc.dram_tensor(name, shape, dt, kind="ExternalInput").ap()

    def dout(name, shape):
        return nc.dram_tensor(name, shape, f32, kind="ExternalOutput").ap()

    io = {
        "xT": din("xT", (FEAT, TOK)),
        "W1": din("W1", (FEAT, F1)),
        "G1": din("G1", (FEAT, F1)),
        "W2": din("W2", (F1, F1), f32r),
        "G2": din("G2", (F1, F1), f32r),
        "W1eK": din("W1eK", (2 * F1, F1), f32r),
        "W2eK": din("W2eK", (F1, F1), f32r),
        "W3m": din("W3m", (F1, F1), f32r),
        "w": din("w", (F1, SZ), f32r),
        "kb1": din("kb1", (F1, 1)),
        "kb2": din("kb2", (F1, 1)),
        "b3": din("b3", (F1, 1)),
        "vT": dout("vT", (F1, TOK)),
        "oT": dout("oT", (SZ, TOK)),
    }

    with tile.TileContext(nc) as tc, ExitStack() as ctx:
        wp = ctx.enter_context(tc.tile_pool(name="wp", bufs=1))
        ap_ = ctx.enter_context(tc.tile_pool(name="act", bufs=1))
        tp = ctx.enter_context(tc.tile_pool(name="tmp", bufs=2))

        def wload(key, n_ktiles, width):
            ts = []
            for k in range(n_ktiles):
                t = wp.tile([P, width], f32r, name=f"{key}_{k}")
                nc.sync.dma_start(t[:], io[key][k * P:(k + 1) * P, :])
                ts.append(t)
            return ts

        def load_x(s):
            row = []
            for k in range(K1):
                xf = tp.tile([P, TS], f32, name=f"xf{k}", tag=f"xf{k}", bufs=2)
                nc.sync.dma_start(xf[:], io["xT"][k * P:(k + 1) * P,
                                                  s * TS:(s + 1) * TS])
                row.append(xf)
            return row

        xpre = [load_x(0), load_x(1)]
        def wsplit(key, n_ktiles):
            hs, ls = [], []
            for k in range(n_ktiles):
                wf = tp.tile([P, F1], f32, name=f"{key}f{k}", tag="wpf", bufs=1)
                nc.sync.dma_start(wf[:], io[key][k * P:(k + 1) * P, :])
                wh = wp.tile([P, F1], f32r, name=f"{key}h_{k}")
                nc.vector.tensor_copy(wh[:], wf[:])
                we_ = tp.tile([P, F1], f32, name=f"{key}e{k}", tag="echo", bufs=1)
                nc.vector.tensor_copy(we_[:], wh[:])
                wl = wp.tile([P, F1], f32r, name=f"{key}l_{k}")
                nc.vector.tensor_sub(wl[:], wf[:], we_[:])
                hs.append(wh)
                ls.append(wl)
            return hs, ls

        def bload(key):
            ts = []
            for m in range(M1):
                t = wp.tile([P, 1], f32, name=f"{key}b_{m}")
                nc.sync.dma_start(t[:], io[key][m * P:(m + 1) * P, :])
                ts.append(t)
            return ts

        kb1t = bload("kb1")
        kb2t = bload("kb2")
        b3t = bload("b3")

        epst = wp.tile([P, 1], f32, name="eps_b")
        nc.gpsimd.memset(epst[:], 1e-7)

        def ms(m):
            return slice(m * P, (m + 1) * P)

        st = [dict() for _ in range(SUP)]

        from concourse.tile import add_dep_helper
        _cur_load = [None]
        _era_ops = []

        def act(*args, **kw):
            bi = nc.scalar.activation(*args, **kw)
            ins = getattr(bi, "ins", bi)
            if _cur_load[0] is not None:
                add_dep_helper(ins, _cur_load[0], sync=False,
                               reason="act-era gate")
            _era_ops.append(ins)
            return bi

        from concourse.hw_specs import get_activation_tables
        _table_names = list(get_activation_tables(nc.m.arch).keys())
        _SET_A = _table_names.index("natural_log_exp_and_others")
        _SET_B = _table_names.index("gelu_apprx_tanh_and_others")

        def era_load(set_id):
            inst = mybir.InstLoadActFuncSet(
                name=nc.get_next_instruction_name(), ins=[], outs=[],
                act_func_set_id=set_id)
            bi = nc.scalar.add_instruction(inst)
            ins = getattr(bi, "ins", bi)
            for prev in _era_ops:
                add_dep_helper(ins, prev, sync=False,
                               reason="act-era barrier")
            _era_ops.clear()
            _cur_load[0] = ins

        # ------------------------------------------------------------------
        def emit_prefetch_x(s):
            st[s]["xf"] = xpre[s] if s < len(xpre) else load_x(s)

        def emit_lnx(s):
            """split x into xr/xl, abs+ln on fp32 x, split lx into lxr/lxl."""
            S = st[s]
            xparts = []
            for k in range(K1):
                xf = S["xf"][k]
                xr = ap_.tile([P, TS], f32r, name=f"xr{k}", tag=f"xr{k}", bufs=1)
                nc.vector.tensor_copy(xr[:], xf[:])
                xe = tp.tile([P, TS], f32, name=f"xec{k}", tag="echo", bufs=1)
                nc.vector.tensor_copy(xe[:], xr[:])
                xl = ap_.tile([P, TS], f32r, name=f"xl{k}", tag=f"xl{k}", bufs=1)
                nc.vector.tensor_sub(xl[:], xf[:], xe[:])
                xparts.append((xf, xr, xl))
            S["xparts"] = xparts
            lxrk, lxlk = [], []
            for k in range(K1):
                xf = S["xparts"][k][0]
                lxf = tp.tile([P, TS], f32, name=f"lxf{k}", tag=f"lxf{k}", bufs=1)
                act(lxf[:], xf[:], AFT.Abs)
                act(lxf[:], lxf[:], AFT.Ln, bias=epst[:])
                lxr = ap_.tile([P, TS], f32r, name=f"lxr{k}", tag=f"lxr{k}",
                               bufs=1)
                nc.vector.tensor_copy(lxr[:], lxf[:])
                le = tp.tile([P, TS], f32, name=f"le{k}", tag="echo", bufs=1)
                nc.vector.tensor_copy(le[:], lxr[:])
                lxl = ap_.tile([P, TS], f32r, name=f"lxl{k}", tag=f"lxl{k}",
                               bufs=1)
                nc.vector.tensor_sub(lxl[:], lxf[:], le[:])
                lxrk.append(lxr)
                lxlk.append(lxl)
            S["lxrk"], S["lxlk"] = lxrk, lxlk

        def l1_terms(hs, ls, parts_r, parts_l):
            """k-sequence for a 3-term split matmul group (6 k-tiles)."""
            seq = []
            for k in range(K1):
                seq.append((hs[k], parts_r[k]))
            for k in range(K1):
                seq.append((hs[k], parts_l[k]))
            for k in range(K1):
                seq.append((ls[k], parts_r[k]))
            return seq

        def era_A_L1(s):
            """ln/exp-table era: a1/p1 split matmuls, exp."""
            S = st[s]
            xr = [pk[1] for pk in S["xparts"]]
            xl = [pk[2] for pk in S["xparts"]]
            with tc.tile_pool(name=f"psA1_{s}", bufs=1, space="PSUM") as pp:
                a1sb, m1c = [], []
                for m in range(M1):
                    pa = pp.tile([P, TS], f32, name=f"a1_{m}", tag="a1", bufs=2)
                    seq = l1_terms(W1h, W1l, xr, xl)
                    for j, (wt_, xt_) in enumerate(seq):
                        nc.tensor.matmul(pa[:], wt_[:, ms(m)], xt_[:],
                                         start=(j == 0), stop=(j == len(seq) - 1))
                    a1 = ap_.tile([P, TS], f32, name=f"a1sb{m}", tag=f"a1sb{m}",
                                  bufs=1)
                    nc.vector.tensor_copy(a1[:], pa[:])
                    a1sb.append(a1)

                    pm = pp.tile([P, TS], f32, name=f"p1_{m}", tag="p1", bufs=2)
                    seq = l1_terms(W1h, W1l, S["lxrk"], S["lxlk"])
                    for j, (wt_, xt_) in enumerate(seq):
                        nc.tensor.matmul(pm[:], wt_[:, ms(m)], xt_[:],
                                         start=(j == 0), stop=(j == len(seq) - 1))
                    mc = ap_.tile([P, TS], f32, name=f"m1c{m}", tag=f"m1c{m}",
                                  bufs=1)
                    act(mc[:], pm[:], AFT.Exp)
                    m1c.append(mc)
                S["a1sb"], S["m1c"] = a1sb, m1c

        # ------------------------------------------------------------------
        def era_B_L1(s):
            """gelu-table era: g1 matmul + tanh gate, combine, h11/h12."""
            S = st[s]
            with tc.tile_pool(name=f"psB1_{s}", bufs=1, space="PSUM") as pp:
                a1sb, m1c = S["a1sb"], S["m1c"]
                # independent ACT work first: h11 gelus, then gate tanhs
                h11 = []
                for m in range(M1):
                    h1 = ap_.tile([P, TS], f32r, name=f"h11_{m}", tag=f"h11_{m}",
                                  bufs=1)
                    act(h1[:], a1sb[m][:], AFT.Gelu_apprx_tanh)
                    h11.append(h1)
                t1s, ds = [], []
                xr = [pk[1] for pk in S["xparts"]]
                xl = [pk[2] for pk in S["xparts"]]
                for m in range(M1):
                    pg = pp.tile([P, TS], f32, name=f"g1_{m}", tag="g1", bufs=2)
                    seq = l1_terms(G1h, G1l, xr, xl)
                    for j, (wt_, xt_) in enumerate(seq):
                        nc.tensor.matmul(pg[:], wt_[:, ms(m)], xt_[:],
                                         start=(j == 0), stop=(j == len(seq) - 1))
                    t1 = tp.tile([P, TS], f32, name=f"t1_{m}", tag=f"tg{m}", bufs=1)
                    act(t1[:], pg[:], AFT.Tanh, scale=0.5)
                    t1s.append(t1)
                    d = tp.tile([P, TS], f32, name=f"d1_{m}", tag="d", bufs=4)
                    nc.vector.tensor_sub(d[:], a1sb[m][:], m1c[m][:])
                    ds.append(d)
                # h12 feeds ln(|.|) downstream, so its gelu must be exact
                # (LUT error near zero explodes through the log). Compute
                # z = gated combine, then qp = z*(1+tanh(c*(z+0.044715 z^3)))
                # = 2*gelu(z) via tanh LUT + DVE; the 0.5 folds into the Ln
                # scale in era_A_L2.
                C1 = float(np.sqrt(2.0 / np.pi))
                C2 = float(np.sqrt(2.0 / np.pi) * 0.044715)
                h12 = []
                for m in range(M1):
                    nc.vector.tensor_mul(t1s[m][:], t1s[m][:], ds[m][:])
                    nc.vector.tensor_add(ds[m][:], ds[m][:], t1s[m][:])
                    nc.vector.scalar_tensor_tensor(ds[m][:], ds[m][:], 0.5,
                                                   m1c[m][:], ALU.mult, ALU.add)
                    z = ds[m]
                    s1 = t1s[m]
                    nc.vector.tensor_mul(s1[:], z[:], z[:])
                    nc.vector.tensor_scalar(s1[:], s1[:], C2, C1, ALU.mult,
                                            ALU.add)
                    nc.vector.tensor_mul(s1[:], s1[:], z[:])
                    act(s1[:], s1[:], AFT.Tanh)
                    h2 = ap_.tile([P, TS], f32, name=f"h12_{m}", tag=f"h12_{m}",
                                  bufs=1)
                    nc.vector.scalar_tensor_tensor(h2[:], s1[:], 1.0, z[:],
                                                   ALU.add, ALU.mult)
                    h12.append(h2)
                S["h11"], S["h12"] = h11, h12

        # ------------------------------------------------------------------
        def era_A_L2(s):
            """ln/exp era for layer-2: lx2, p2/a2/g2 matmuls, exp, clip, evacs."""
            S = st[s]
            h11, h12 = S["h11"], S["h12"]
            with tc.tile_pool(name=f"psA2_{s}", bufs=1, space="PSUM") as pp:
                # era-agnostic PE work first so its DVE evacs are ready for era B
                a2sb, g2sb = [], []
                for m in range(M1):
                    pa = pp.tile([P, TS], f32, name=f"a2_{m}", tag="a2", bufs=1)
                    for k in range(K2):
                        nc.tensor.matmul(pa[:], W2t[k][:, ms(m)], h11[k][:],
                                         start=(k == 0), stop=(k == K2 - 1))
                    a2 = ap_.tile([P, TS], f32r, name=f"a2sb{m}", tag=f"a2sb{m}",
                                  bufs=1)
                    nc.vector.tensor_copy(a2[:], pa[:])
                    a2sb.append(a2)

                    pg = pp.tile([P, TS], f32, name=f"g2_{m}", tag="g2", bufs=1)
                    for k in range(K2):
                        nc.tensor.matmul(pg[:], G2t[k][:, ms(m)], h11[k][:],
                                         start=(k == 0), stop=(k == K2 - 1))
                    gv = ap_.tile([P, TS], f32, name=f"g2sb{m}", tag=f"g2sb{m}",
                                  bufs=1)
                    nc.vector.tensor_copy(gv[:], pg[:])
                    g2sb.append(gv)
                lx2 = []
                for k in range(K2):
                    l2 = ap_.tile([P, TS], f32r, name=f"lx2_{k}", tag=f"lx2_{k}",
                                  bufs=1)
                    act(l2[:], h12[k][:], AFT.Abs)
                    act(l2[:], l2[:], AFT.Ln, bias=epst[:], scale=0.5)
                    lx2.append(l2)
                m2c = []
                for m in range(M1):
                    pm = pp.tile([P, TS], f32, name=f"p2_{m}", tag="p2", bufs=2)
                    for k in range(K2):
                        nc.tensor.matmul(pm[:], W2t[k][:, ms(m)], lx2[k][:],
                                         start=(k == 0), stop=(k == K2 - 1))
                    mc = ap_.tile([P, TS], f32r, name=f"m2c{m}", tag=f"m2c{m}",
                                  bufs=1)
                    act(mc[:], pm[:], AFT.Exp)
                    m2c.append(mc)
                S["m2c"], S["a2sb"], S["g2sb"] = m2c, a2sb, g2sb

        # ------------------------------------------------------------------
        def era_B_L2comb(s):
            """gelu era: layer-2 combine; ha/ht overwrite a2sb/m2c in place."""
            S = st[s]
            a2sb, m2c, g2sb = S["a2sb"], S["m2c"], S["g2sb"]
            t2s, ds = [], []
            for m in range(M1):
                t2 = tp.tile([P, TS], f32, name=f"t2_{m}", tag=f"tg{m}", bufs=1)
                act(t2[:], g2sb[m][:], AFT.Tanh, scale=0.5)
                t2s.append(t2)
                d = tp.tile([P, TS], f32, name=f"d2_{m}", tag="d", bufs=4)
                nc.vector.tensor_sub(d[:], a2sb[m][:].bitcast(f32),
                                     m2c[m][:].bitcast(f32))
                ds.append(d)
            ha = []
            for m in range(M1):
                # gelu(a2) written over the (f32r) a2sb tile in place
                act(a2sb[m][:], a2sb[m][:].bitcast(f32), AFT.Gelu_apprx_tanh)
                ha.append(a2sb[m])
            ht = []
            for m in range(M1):
                nc.vector.tensor_mul(t2s[m][:], t2s[m][:], ds[m][:])
                nc.vector.tensor_add(ds[m][:], ds[m][:], t2s[m][:])
                nc.vector.scalar_tensor_tensor(ds[m][:], ds[m][:], 0.5,
                                               m2c[m][:].bitcast(f32),
                                               ALU.mult, ALU.add)
                act(m2c[m][:], ds[m][:], AFT.Gelu_apprx_tanh)
                ht.append(m2c[m])
            S["ha"], S["ht"] = ha, ht

        def era_fc(s):
            """gelu era: folded fc chain, v and o out (runs one period late)."""
            S = st[s]
            tsl = slice(s * TS, (s + 1) * TS)
            ha, ht = S["ha"], S["ht"]
            with tc.tile_pool(name=f"psFC_{s}", bufs=1, space="PSUM") as pp:
                # fc1e+Keff folded: xa1 = gelu([ha;ht] @ (W1e@Keff) + b1e@Keff)
                xa1 = []
                for m in range(M1):
                    pz = pp.tile([P, TS], f32, name=f"z1_{m}", tag="z1", bufs=3)
                    for k in range(KE):
                        srck = ha[k] if k < K2 else ht[k - K2]
                        nc.tensor.matmul(pz[:], W1eKt[k][:, ms(m)], srck[:],
                                         start=(k == 0), stop=(k == KE - 1))
                    x1 = ap_.tile([P, TS], f32r, name=f"xa1_{m}", tag=f"xe_{m}",
                                  bufs=2)
                    act(x1[:], pz[:], AFT.Gelu_apprx_tanh,
                                         bias=kb1t[m][:])
                    xa1.append(x1)
                # o = xa1 @ w
                for mo in range(MO):
                    mw = P if mo == 0 else SZ - P
                    po = pp.tile([mw, TS], f32, name=f"zo_{mo}", tag="zx", bufs=3)
                    for k in range(K2):
                        nc.tensor.matmul(po[:], wt[k][:, mo * P:mo * P + mw],
                                         xa1[k][:],
                                         start=(k == 0), stop=(k == K2 - 1))
                    oo = tp.tile([mw, TS], f32, name=f"oo_{mo}", tag="outb", bufs=2)
                    nc.vector.tensor_copy(oo[:], po[:])
                    nc.sync.dma_start(io["oT"][mo * P:mo * P + mw, tsl], oo[:])
                # fc2e+Keff folded: xa2 = gelu(xa1 @ (W2e@Keff) + b2e@Keff)
                xa2 = []
                for m in range(M1):
                    pz = pp.tile([P, TS], f32, name=f"z2_{m}", tag="zx", bufs=3)
                    for k in range(K2):
                        nc.tensor.matmul(pz[:], W2eKt[k][:, ms(m)], xa1[k][:],
                                         start=(k == 0), stop=(k == K2 - 1))
                    x2 = ap_.tile([P, TS], f32r, name=f"xa2_{m}", tag=f"xe_{m}",
                                  bufs=2)
                    act(x2[:], pz[:], AFT.Gelu_apprx_tanh,
                                         bias=kb2t[m][:])
                    xa2.append(x2)
                # fc3: v = xa2 @ W3m + b3
                for m in range(M1):
                    pz = pp.tile([P, TS], f32, name=f"zv_{m}", tag="zx", bufs=3)
                    for k in range(K2):
                        nc.tensor.matmul(pz[:], W3mt[k][:, ms(m)], xa2[k][:],
                                         start=(k == 0), stop=(k == K2 - 1))
                    vo = tp.tile([P, TS], f32, name=f"vo_{m}", tag="outb", bufs=2)
                    nc.vector.tensor_scalar(vo[:], pz[:], b3t[m][:], None, ALU.add)
                    nc.sync.dma_start(io["vT"][m * P:(m + 1) * P, tsl], vo[:])

        # ---- staggered era schedule: L1(s) | L2(s-1) ----
        era_load(_SET_A)
        emit_prefetch_x(0)
        emit_prefetch_x(1)
        emit_lnx(0)
        W1h, W1l = wsplit("W1", K1)
        G1h, G1l = wsplit("G1", K1)
        W2t = wload("W2", K2, F1)
        G2t = wload("G2", K2, F1)
        W1eKt = wload("W1eK", KE, F1)
        W2eKt = wload("W2eK", K2, F1)
        W3mt = wload("W3m", K2, F1)
        wt = wload("w", K2, SZ)
        era_A_L1(0)
        era_load(_SET_B)
        era_B_L1(0)
        for s in range(1, SUP):
            era_load(_SET_A)
            if s + 1 < SUP:
                emit_prefetch_x(s + 1)
            emit_lnx(s)
            era_A_L2(s - 1)
            era_A_L1(s)
            era_load(_SET_B)
            era_B_L2comb(s - 1)
            era_B_L1(s)
            era_fc(s - 1)
        era_load(_SET_A)
        era_A_L2(SUP - 1)
        era_load(_SET_B)
        era_B_L2comb(SUP - 1)
        era_fc(SUP - 1)

    nc.compile()
    return nc


def _prep_host(inputs):
    f = np.asarray(inputs["features"], np.float32).reshape(NTOK, FEAT)
    W_hat1 = np.asarray(inputs["W_hat1"], np.float32)
    M_hat1 = np.asarray(inputs["M_hat1"], np.float32)
    G1 = np.asarray(inputs["G1"], np.float32)
    W_hat2 = np.asarray(inputs["W_hat2"], np.float32)
    M_hat2 = np.asarray(inputs["M_hat2"], np.float32)
    G2 = np.asarray(inputs["G2"], np.float32)
    W1e = np.asarray(inputs["W1e"], np.float32)
    b1e = np.asarray(inputs["b1e"], np.float32)
    W2e = np.asarray(inputs["W2e"], np.float32)
    b2e = np.asarray(inputs["b2e"], np.float32)
    W3 = np.asarray(inputs["W3"], np.float32)
    b3 = np.asarray(inputs["b3"], np.float32)
    kn = np.asarray(inputs["kn"], np.float32)
    w = np.asarray(inputs["w"], np.float32)
    we = np.asarray(inputs["we"], np.float32)
    sz_mini = int(inputs["sz_mini"])

    # weight-only preparation (static per model)
    W1 = (np.tanh(W_hat1) * (1.0 / (1.0 + np.exp(-M_hat1)))).astype(np.float32)
    W2 = (np.tanh(W_hat2) * (1.0 / (1.0 + np.exp(-M_hat2)))).astype(np.float32)
    Keff = (np.kron(np.eye(sz_mini, dtype=np.float32), kn)[:F1, :F1] * we)
    Keff64 = Keff.astype(np.float64)
    # fold the (linear) Keff matmul into the preceding dense weights:
    # (h @ W1e + b1e) @ Keff == h @ (W1e@Keff) + (b1e@Keff)
    W1eK = np.ascontiguousarray((W1e.astype(np.float64) @ Keff64).astype(np.float32))
    W2eK = np.ascontiguousarray((W2e.astype(np.float64) @ Keff64).astype(np.float32))
    W3m = np.ascontiguousarray(W3 * we, np.float32)
    kb1 = (b1e.astype(np.float64) @ Keff64).astype(np.float32).reshape(F1, 1)
    kb2 = (b2e.astype(np.float64) @ Keff64).astype(np.float32).reshape(F1, 1)
    b3c = b3.astype(np.float32).reshape(F1, 1)

    xT = np.ascontiguousarray(f.T)  # [FEAT, NTOK]

    shared = {
        "W1": W1, "G1": np.ascontiguousarray(G1),
        "W2": W2, "G2": np.ascontiguousarray(G2),
        "W1eK": W1eK, "W2eK": W2eK, "W3m": W3m,
        "w": np.ascontiguousarray(w),
        "kb1": kb1, "kb2": kb2, "b3": b3c,
    }
    in_maps = []
    for c in range(NCORES):
        im = dict(shared)
        im["xT"] = np.ascontiguousarray(xT[:, c * TOK:(c + 1) * TOK])
        in_maps.append(im)
    return in_maps


def kernel(**inputs):
    from concourse.bass_utils import run_bass_kernel_spmd

    if "nc" not in _CACHE:
        _CACHE["nc"] = _build_program()
    nc = _CACHE["nc"]

    in_maps = _prep_host(inputs)
    res = run_bass_kernel_spmd(nc, in_maps, core_ids=list(range(NCORES)))

    v = np.empty((NTOK, F1), np.float32)
    o = np.empty((NTOK, SZ), np.float32)
    for c in range(NCORES):
        rr = res.results[c]
        v[c * TOK:(c + 1) * TOK, :] = rr["vT"].T
        o[c * TOK:(c + 1) * TOK, :] = rr["oT"].T
    o = o.reshape(B, T, ACTION_DIM, ATOMS)
    v = v.reshape(B, T, F1)
    return o, v


# revision 46
# speedup vs baseline: 1.1612x; 1.1612x over previous
"""Trainium2 Bass kernel for nn_ActorMultiEnv_v2 (NALU actor network).

Strategy:
  - Data-parallel: 16384 tokens sharded as 2048 tokens per core across 8 cores.
  - Feature-major on-chip layout: activations stored [feature(partition), token(free)],
    so every matmul uses the weight in natural [in, out] layout as the stationary
    operand and never needs an on-device transpose.
  - All matmuls run as float32r (full fp32 data, reduced-precision multiplier,
    1 cycle/row on the PE -- 4x faster than true fp32).
  - sigmoid(g) is computed as 0.5 + 0.5*tanh(g/2) so the whole network needs only
    two ACT table sets (natural_log_exp and gelu_apprx_tanh); token superchunks are
    staggered through the two table "eras" to keep table swaps ~O(10).
  - Host does weight-only preparation (tanh(W_hat)*sigmoid(M_hat), the block-diagonal
    Keff = kron(I, kn)[:f1,:f1]*we, W3*we, bias folds) and input/output layout
    (transpose to feature-major shards, gather + transpose back).
"""

import sys

import numpy as np

for _p in ("/opt/trn_rl_repo",):
    if _p not in sys.path:
        sys.path.append(_p)

# ---- problem constants (hardcoded per harness contract) ----
B, T, FEAT, F1 = 128, 128, 256, 512
ACTION_DIM, ATOMS = 6, 32
SZ = ACTION_DIM * ATOMS            # 192
NCORES = 8
NTOK = B * T                        # 16384
TOK = NTOK // NCORES                # 2048 tokens per core
SUP = 4                             # superchunks per core
TS = TOK // SUP                     # 512 tokens per superchunk (== one PSUM chunk)
P = 128
M1 = F1 // P                        # 4 output-feature tiles
K1 = FEAT // P                      # 2 input k-tiles (layer 1)
K2 = F1 // P                        # 4 k-tiles (f1-sized contractions)
KE = 2 * F1 // P                    # 8 k-tiles (fc1e)
MO = 2                              # o-matmul M tiles (128 + 64)

EXP_LO = float(np.exp(np.float64(np.float32(-20.0))))
EXP_HI = float(np.exp(np.float64(np.float32(20.0))))

_CACHE = {}


def _build_program():
    from contextlib import ExitStack

    import concourse.tile as tile
    from concourse import bacc, mybir

    f32 = mybir.dt.float32
    f32r = mybir.dt.float32r
    AFT = mybir.ActivationFunctionType
    ALU = mybir.AluOpType

    nc = bacc.Bacc("TRN2", target_bir_lowering=False, debug=False,
                   num_devices=NCORES)

    def din(name, shape, dt=f32):
        return nc.dram_tensor(name, shape, dt, kind="ExternalInput").ap()

    def dout(name, shape):
        return nc.dram_tensor(name, shape, f32, kind="ExternalOutput").ap()

    io = {
        "xT": din("xT", (FEAT, TOK)),
        "W1": din("W1", (FEAT, F1)),
        "G1": din("G1", (FEAT, F1)),
        "W2": din("W2", (F1, F1), f32r),
        "G2": din("G2", (F1, F1), f32r),
        "W1eK": din("W1eK", (2 * F1, F1), f32r),
        "W2eK": din("W2eK", (F1, F1), f32r),
        "W3m": din("W3m", (F1, F1), f32r),
        "w": din("w", (F1, SZ), f32r),
        "kb1": din("kb1", (F1, 1)),
        "kb2": din("kb2", (F1, 1)),
        "b3": din("b3", (F1, 1)),
        "vT": dout("vT", (F1, TOK)),
        "oT": dout("oT", (SZ, TOK)),
    }

    with tile.TileContext(nc) as tc, ExitStack() as ctx:
        wp = ctx.enter_context(tc.tile_pool(name="wp", bufs=1))
        ap_ = ctx.enter_context(tc.tile_pool(name="act", bufs=1))
        tp = ctx.enter_context(tc.tile_pool(name="tmp", bufs=2))

        def wload(key, n_ktiles, width):
            ts = []
            for k in range(n_ktiles):
                t = wp.tile([P, width], f32r, name=f"{key}_{k}")
                nc.sync.dma_start(t[:], io[key][k * P:(k + 1) * P, :])
                ts.append(t)
            return ts

        def load_x(s):
            row = []
            for k in range(K1):
                xf = tp.tile([P, TS], f32, name=f"xf{k}", tag=f"xf{k}", bufs=2)
                nc.sync.dma_start(xf[:], io["xT"][k * P:(k + 1) * P,
                                                  s * TS:(s + 1) * TS])
                row.append(xf)
            return row

        xpre = [load_x(0), load_x(1)]
        def wsplit(key, n_ktiles):
            hs, ls = [], []
            for k in range(n_ktiles):
                wf = tp.tile([P, F1], f32, name=f"{key}f{k}", tag="wpf", bufs=1)
                nc.sync.dma_start(wf[:], io[key][k * P:(k + 1) * P, :])
                wh = wp.tile([P, F1], f32r, name=f"{key}h_{k}")
                nc.vector.tensor_copy(wh[:], wf[:])
                we_ = tp.tile([P, F1], f32, name=f"{key}e{k}", tag="echo", bufs=1)
                nc.vector.tensor_copy(we_[:], wh[:])
                wl = wp.tile([P, F1], f32r, name=f"{key}l_{k}")
                nc.vector.tensor_sub(wl[:], wf[:], we_[:])
                hs.append(wh)
                ls.append(wl)
            return hs, ls

        def bload(key):
            ts = []
            for m in range(M1):
                t = wp.tile([P, 1], f32, name=f"{key}b_{m}")
                nc.sync.dma_start(t[:], io[key][m * P:(m + 1) * P, :])
                ts.append(t)
            return ts

        kb1t = bload("kb1")
        kb2t = bload("kb2")
        b3t = bload("b3")

        epst = wp.tile([P, 1], f32, name="eps_b")
        nc.gpsimd.memset(epst[:], 1e-7)

        def ms(m):
            return slice(m * P, (m + 1) * P)

        st = [dict() for _ in range(SUP)]

        from concourse.tile import add_dep_helper
        _cur_load = [None]
        _era_ops = []

        def act(*args, **kw):
            bi = nc.scalar.activation(*args, **kw)
            ins = getattr(bi, "ins", bi)
            if _cur_load[0] is not None:
                add_dep_helper(ins, _cur_load[0], sync=False,
                               reason="act-era gate")
            _era_ops.append(ins)
            return bi

        from concourse.hw_specs import get_activation_tables
        _table_names = list(get_activation_tables(nc.m.arch).keys())
        _SET_A = _table_names.index("natural_log_exp_and_others")
        _SET_B = _table_names.index("gelu_apprx_tanh_and_others")

        def era_load(set_id):
            inst = mybir.InstLoadActFuncSet(
                name=nc.get_next_instruction_name(), ins=[], outs=[],
                act_func_set_id=set_id)
            bi = nc.scalar.add_instruction(inst)
            ins = getattr(bi, "ins", bi)
            for prev in _era_ops:
                add_dep_helper(ins, prev, sync=False,
                               reason="act-era barrier")
            _era_ops.clear()
            _cur_load[0] = ins

        # ------------------------------------------------------------------
        def emit_prefetch_x(s):
            st[s]["xf"] = xpre[s] if s < len(xpre) else load_x(s)

        def emit_lnx(s):
            """split x into xr/xl, abs+ln on fp32 x, split lx into lxr/lxl."""
            S = st[s]
            xparts = []
            for k in range(K1):
                xf = S["xf"][k]
                xr = ap_.tile([P, TS], f32r, name=f"xr{k}", tag=f"xr{k}", bufs=1)
                nc.vector.tensor_copy(xr[:], xf[:])
                xe = tp.tile([P, TS], f32, name=f"xec{k}", tag="echo", bufs=1)
                nc.vector.tensor_copy(xe[:], xr[:])
                xl = ap_.tile([P, TS], f32r, name=f"xl{k}", tag=f"xl{k}", bufs=1)
                nc.vector.tensor_sub(xl[:], xf[:], xe[:])
                xparts.append((xf, xr, xl))
            S["xparts"] = xparts
            lxrk, lxlk = [], []
            for k in range(K1):
                xf = S["xparts"][k][0]
                lxf = tp.tile([P, TS], f32, name=f"lxf{k}", tag=f"lxf{k}", bufs=1)
                act(lxf[:], xf[:], AFT.Abs)
                act(lxf[:], lxf[:], AFT.Ln, bias=epst[:])
                lxr = ap_.tile([P, TS], f32r, name=f"lxr{k}", tag=f"lxr{k}",
                               bufs=1)
                nc.vector.tensor_copy(lxr[:], lxf[:])
                le = tp.tile([P, TS], f32, name=f"le{k}", tag="echo", bufs=1)
                nc.vector.tensor_copy(le[:], lxr[:])
                lxl = ap_.tile([P, TS], f32r, name=f"lxl{k}", tag=f"lxl{k}",
                               bufs=1)
                nc.vector.tensor_sub(lxl[:], lxf[:], le[:])
                lxrk.append(lxr)
                lxlk.append(lxl)
            S["lxrk"], S["lxlk"] = lxrk, lxlk

        def l1_terms(hs, ls, parts_r, parts_l):
            """k-sequence for a 3-term split matmul group (6 k-tiles)."""
            seq = []
            for k in range(K1):
                seq.append((hs[k], parts_r[k]))
            for k in range(K1):
                seq.append((hs[k], parts_l[k]))
            for k in range(K1):
                seq.append((ls[k], parts_r[k]))
            return seq

        def era_A_L1(s):
            """ln/exp-table era: a1/p1 split matmuls, exp."""
            S = st[s]
            xr = [pk[1] for pk in S["xparts"]]
            xl = [pk[2] for pk in S["xparts"]]
            with tc.tile_pool(name=f"psA1_{s}", bufs=1, space="PSUM") as pp:
                a1sb, m1c = [], []
                for m in range(M1):
                    pa = pp.tile([P, TS], f32, name=f"a1_{m}", tag="a1", bufs=2)
                    seq = l1_terms(W1h, W1l, xr, xl)
                    for j, (wt_, xt_) in enumerate(seq):
                        nc.tensor.matmul(pa[:], wt_[:, ms(m)], xt_[:],
                                         start=(j == 0), stop=(j == len(seq) - 1))
                    a1 = ap_.tile([P, TS], f32, name=f"a1sb{m}", tag=f"a1sb{m}",
                                  bufs=1)
                    nc.vector.tensor_copy(a1[:], pa[:])
                    a1sb.append(a1)

                    pm = pp.tile([P, TS], f32, name=f"p1_{m}", tag="p1", bufs=2)
                    seq = l1_terms(W1h, W1l, S["lxrk"], S["lxlk"])
                    for j, (wt_, xt_) in enumerate(seq):
                        nc.tensor.matmul(pm[:], wt_[:, ms(m)], xt_[:],
                                         start=(j == 0), stop=(j == len(seq) - 1))
                    mc = ap_.tile([P, TS], f32, name=f"m1c{m}", tag=f"m1c{m}",
                                  bufs=1)
                    act(mc[:], pm[:], AFT.Exp)
                    m1c.append(mc)
                S["a1sb"], S["m1c"] = a1sb, m1c

        # ------------------------------------------------------------------
        def era_B_L1(s):
            """gelu-table era: g1 matmul + tanh gate, combine, h11/h12."""
            S = st[s]
            with tc.tile_pool(name=f"psB1_{s}", bufs=1, space="PSUM") as pp:
                a1sb, m1c = S["a1sb"], S["m1c"]
                # independent ACT work first: h11 gelus, then gate tanhs
                h11 = []
                for m in range(M1):
                    h1 = ap_.tile([P, TS], f32r, name=f"h11_{m}", tag=f"h11_{m}",
                                  bufs=1)
                    act(h1[:], a1sb[m][:], AFT.Gelu_apprx_tanh)
                    h11.append(h1)
                t1s, ds = [], []
                xr = [pk[1] for pk in S["xparts"]]
                xl = [pk[2] for pk in S["xparts"]]
                for m in range(M1):
                    pg = pp.tile([P, TS], f32, name=f"g1_{m}", tag="g1", bufs=2)
                    seq = l1_terms(G1h, G1l, xr, xl)
                    for j, (wt_, xt_) in enumerate(seq):
                        nc.tensor.matmul(pg[:], wt_[:, ms(m)], xt_[:],
                                         start=(j == 0), stop=(j == len(seq) - 1))
                    t1 = tp.tile([P, TS], f32, name=f"t1_{m}", tag=f"tg{m}", bufs=1)
                    act(t1[:], pg[:], AFT.Tanh, scale=0.5)
                    t1s.append(t1)
                    d = tp.tile([P, TS], f32, name=f"d1_{m}", tag="d", bufs=4)
                    nc.vector.tensor_sub(d[:], a1sb[m][:], m1c[m][:])
                    ds.append(d)
                # h12 feeds ln(|.|) downstream, so its gelu must be exact
                # (LUT error near zero explodes through the log). Compute
                # z = gated combine, then qp = z*(1+tanh(c*(z+0.044715 z^3)))
                # = 2*gelu(z) via tanh LUT + DVE; the 0.5 folds into the Ln
                # scale in era_A_L2.
                C1 = float(np.sqrt(2.0 / np.pi))
                C2 = float(np.sqrt(2.0 / np.pi) * 0.044715)
                h12 = []
                for m in range(M1):
                    nc.vector.tensor_mul(t1s[m][:], t1s[m][:], ds[m][:])
                    nc.vector.tensor_add(ds[m][:], ds[m][:], t1s[m][:])
                    nc.vector.scalar_tensor_tensor(ds[m][:], ds[m][:], 0.5,
                                                   m1c[m][:], ALU.mult, ALU.add)
                    z = ds[m]
                    s1 = t1s[m]
                    nc.vector.tensor_mul(s1[:], z[:], z[:])
                    nc.vector.tensor_scalar(s1[:], s1[:], C2, C1, ALU.mult,
                                            ALU.add)
                    nc.vector.tensor_mul(s1[:], s1[:], z[:])
                    act(s1[:], s1[:], AFT.Tanh)
                    h2 = ap_.tile([P, TS], f32, name=f"h12_{m}", tag=f"h12_{m}",
                                  bufs=1)
                    nc.vector.scalar_tensor_tensor(h2[:], s1[:], 1.0, z[:],
                                                   ALU.add, ALU.mult)
                    h12.append(h2)
                S["h11"], S["h12"] = h11, h12

        # ------------------------------------------------------------------
        def era_A_L2(s):
            """ln/exp era for layer-2: lx2, p2/a2/g2 matmuls, exp, clip, evacs."""
            S = st[s]
            h11, h12 = S["h11"], S["h12"]
            with tc.tile_pool(name=f"psA2_{s}", bufs=1, space="PSUM") as pp:
                # era-agnostic PE work first so its DVE evacs are ready for era B
                a2sb, g2sb = [], []
                for m in range(M1):
                    pa = pp.tile([P, TS], f32, name=f"a2_{m}", tag="a2", bufs=1)
                    for k in range(K2):
                        nc.tensor.matmul(pa[:], W2t[k][:, ms(m)], h11[k][:],
                                         start=(k == 0), stop=(k == K2 - 1))
                    a2 = ap_.tile([P, TS], f32r, name=f"a2sb{m}", tag=f"a2sb{m}",
                                  bufs=1)
                    nc.vector.tensor_copy(a2[:], pa[:])
                    a2sb.append(a2)

                    pg = pp.tile([P, TS], f32, name=f"g2_{m}", tag="g2", bufs=1)
                    for k in range(K2):
                        nc.tensor.matmul(pg[:], G2t[k][:, ms(m)], h11[k][:],
                                         start=(k == 0), stop=(k == K2 - 1))
                    gv = ap_.tile([P, TS], f32, name=f"g2sb{m}", tag=f"g2sb{m}",
                                  bufs=1)
                    nc.vector.tensor_copy(gv[:], pg[:])
                    g2sb.append(gv)
                lx2 = []
                for k in range(K2):
                    l2 = ap_.tile([P, TS], f32r, name=f"lx2_{k}", tag=f"lx2_{k}",
                                  bufs=1)
                    act(l2[:], h12[k][:], AFT.Abs)
                    act(l2[:], l2[:], AFT.Ln, bias=epst[:], scale=0.5)
                    lx2.append(l2)
                m2c = []
                for m in range(M1):
                    pm = pp.tile([P, TS], f32, name=f"p2_{m}", tag="p2", bufs=2)
                    for k in range(K2):
                        nc.tensor.matmul(pm[:], W2t[k][:, ms(m)], lx2[k][:],
                                         start=(k == 0), stop=(k == K2 - 1))
                    mc = ap_.tile([P, TS], f32r, name=f"m2c{m}", tag=f"m2c{m}",
                                  bufs=1)
                    act(mc[:], pm[:], AFT.Exp)
                    m2c.append(mc)
                S["m2c"], S["a2sb"], S["g2sb"] = m2c, a2sb, g2sb

        # ------------------------------------------------------------------
        def era_B_L2comb(s):
            """gelu era: layer-2 combine; ha/ht overwrite a2sb/m2c in place."""
            S = st[s]
            a2sb, m2c, g2sb = S["a2sb"], S["m2c"], S["g2sb"]
            t2s, ds = [], []
            for m in range(M1):
                t2 = tp.tile([P, TS], f32, name=f"t2_{m}", tag=f"tg{m}", bufs=1)
                act(t2[:], g2sb[m][:], AFT.Tanh, scale=0.5)
                t2s.append(t2)
                d = tp.tile([P, TS], f32, name=f"d2_{m}", tag="d", bufs=4)
                nc.vector.tensor_sub(d[:], a2sb[m][:].bitcast(f32),
                                     m2c[m][:].bitcast(f32))
                ds.append(d)
            ha = []
            for m in range(M1):
                # gelu(a2) written over the (f32r) a2sb tile in place
                act(a2sb[m][:], a2sb[m][:].bitcast(f32), AFT.Gelu_apprx_tanh)
                ha.append(a2sb[m])
            ht = []
            for m in range(M1):
                nc.vector.tensor_mul(t2s[m][:], t2s[m][:], ds[m][:])
                nc.vector.tensor_add(ds[m][:], ds[m][:], t2s[m][:])
                nc.vector.scalar_tensor_tensor(ds[m][:], ds[m][:], 0.5,
                                               m2c[m][:].bitcast(f32),
                                               ALU.mult, ALU.add)
                act(m2c[m][:], ds[m][:], AFT.Gelu_apprx_tanh)
                ht.append(m2c[m])
            S["ha"], S["ht"] = ha, ht

        def era_fc(s):
            """gelu era: folded fc chain, v and o out (runs one period late)."""
            S = st[s]
            tsl = slice(s * TS, (s + 1) * TS)
            ha, ht = S["ha"], S["ht"]
            with tc.tile_pool(name=f"psFC_{s}", bufs=1, space="PSUM") as pp:
                # fc1e+Keff folded: xa1 = gelu([ha;ht] @ (W1e@Keff) + b1e@Keff)
                xa1 = []
                for m in range(M1):
                    pz = pp.tile([P, TS], f32, name=f"z1_{m}", tag="z1", bufs=2)
                    for k in range(KE):
                        srck = ha[k] if k < K2 else ht[k - K2]
                        nc.tensor.matmul(pz[:], W1eKt[k][:, ms(m)], srck[:],
                                         start=(k == 0), stop=(k == KE - 1))
                    x1 = ap_.tile([P, TS], f32r, name=f"xa1_{m}", tag=f"xe_{m}",
                                  bufs=2)
                    act(x1[:], pz[:], AFT.Gelu_apprx_tanh,
                                         bias=kb1t[m][:])
                    xa1.append(x1)
                # o = xa1 @ w
                for mo in range(MO):
                    mw = P if mo == 0 else SZ - P
                    po = pp.tile([mw, TS], f32, name=f"zo_{mo}", tag="zx", bufs=2)
                    for k in range(K2):
                        nc.tensor.matmul(po[:], wt[k][:, mo * P:mo * P + mw],
                                         xa1[k][:],
                                         start=(k == 0), stop=(k == K2 - 1))
                    oo = tp.tile([mw, TS], f32, name=f"oo_{mo}", tag="outb", bufs=2)
                    nc.vector.tensor_copy(oo[:], po[:])
                    nc.sync.dma_start(io["oT"][mo * P:mo * P + mw, tsl], oo[:])
                # fc2e+Keff folded: xa2 = gelu(xa1 @ (W2e@Keff) + b2e@Keff)
                xa2 = []
                for m in range(M1):
                    pz = pp.tile([P, TS], f32, name=f"z2_{m}", tag="zx", bufs=2)
                    for k in range(K2):
                        nc.tensor.matmul(pz[:], W2eKt[k][:, ms(m)], xa1[k][:],
                                         start=(k == 0), stop=(k == K2 - 1))
                    x2 = ap_.tile([P, TS], f32r, name=f"xa2_{m}", tag=f"xe_{m}",
                                  bufs=2)
                    act(x2[:], pz[:], AFT.Gelu_apprx_tanh,
                                         bias=kb2t[m][:])
                    xa2.append(x2)
                # fc3: v = xa2 @ W3m + b3
                for m in range(M1):
                    pz = pp.tile([P, TS], f32, name=f"zv_{m}", tag="zx", bufs=2)
                    for k in range(K2):
                        nc.tensor.matmul(pz[:], W3mt[k][:, ms(m)], xa2[k][:],
                                         start=(k == 0), stop=(k == K2 - 1))
                    vo = tp.tile([P, TS], f32, name=f"vo_{m}", tag="outb", bufs=2)
                    nc.vector.tensor_scalar(vo[:], pz[:], b3t[m][:], None, ALU.add)
                    nc.sync.dma_start(io["vT"][m * P:(m + 1) * P, tsl], vo[:])

        # ---- staggered era schedule: L1(s) | L2(s-1) ----
        era_load(_SET_A)
        emit_prefetch_x(0)
        emit_prefetch_x(1)
        emit_lnx(0)
        W1h, W1l = wsplit("W1", K1)
        G1h, G1l = wsplit("G1", K1)
        W2t = wload("W2", K2, F1)
        G2t = wload("G2", K2, F1)
        W1eKt = wload("W1eK", KE, F1)
        W2eKt = wload("W2eK", K2, F1)
        W3mt = wload("W3m", K2, F1)
        wt = wload("w", K2, SZ)
        era_A_L1(0)
        era_load(_SET_B)
        era_B_L1(0)
        for s in range(1, SUP):
            era_load(_SET_A)
            if s + 1 < SUP:
                emit_prefetch_x(s + 1)
            emit_lnx(s)
            era_A_L2(s - 1)
            era_A_L1(s)
            era_load(_SET_B)
            era_B_L2comb(s - 1)
            era_B_L1(s)
            era_fc(s - 1)
        era_load(_SET_A)
        era_A_L2(SUP - 1)
        era_load(_SET_B)
        era_B_L2comb(SUP - 1)
        era_fc(SUP - 1)

    nc.compile()
    return nc


def _prep_host(inputs):
    f = np.asarray(inputs["features"], np.float32).reshape(NTOK, FEAT)
    W_hat1 = np.asarray(inputs["W_hat1"], np.float32)
    M_hat1 = np.asarray(inputs["M_hat1"], np.float32)
    G1 = np.asarray(inputs["G1"], np.float32)
    W_hat2 = np.asarray(inputs["W_hat2"], np.float32)
    M_hat2 = np.asarray(inputs["M_hat2"], np.float32)
    G2 = np.asarray(inputs["G2"], np.float32)
    W1e = np.asarray(inputs["W1e"], np.float32)
    b1e = np.asarray(inputs["b1e"], np.float32)
    W2e = np.asarray(inputs["W2e"], np.float32)
    b2e = np.asarray(inputs["b2e"], np.float32)
    W3 = np.asarray(inputs["W3"], np.float32)
    b3 = np.asarray(inputs["b3"], np.float32)
    kn = np.asarray(inputs["kn"], np.float32)
    w = np.asarray(inputs["w"], np.float32)
    we = np.asarray(inputs["we"], np.float32)
    sz_mini = int(inputs["sz_mini"])

    # weight-only preparation (static per model)
    W1 = (np.tanh(W_hat1) * (1.0 / (1.0 + np.exp(-M_hat1)))).astype(np.float32)
    W2 = (np.tanh(W_hat2) * (1.0 / (1.0 + np.exp(-M_hat2)))).astype(np.float32)
    Keff = (np.kron(np.eye(sz_mini, dtype=np.float32), kn)[:F1, :F1] * we)
    Keff64 = Keff.astype(np.float64)
    # fold the (linear) Keff matmul into the preceding dense weights:
    # (h @ W1e + b1e) @ Keff == h @ (W1e@Keff) + (b1e@Keff)
    W1eK = np.ascontiguousarray((W1e.astype(np.float64) @ Keff64).astype(np.float32))
    W2eK = np.ascontiguousarray((W2e.astype(np.float64) @ Keff64).astype(np.float32))
    W3m = np.ascontiguousarray(W3 * we, np.float32)
    kb1 = (b1e.astype(np.float64) @ Keff64).astype(np.float32).reshape(F1, 1)
    kb2 = (b2e.astype(np.float64) @ Keff64).astype(np.float32).reshape(F1, 1)
    b3c = b3.astype(np.float32).reshape(F1, 1)

    xT = np.ascontiguousarray(f.T)  # [FEAT, NTOK]

    shared = {
        "W1": W1, "G1": np.ascontiguousarray(G1),
        "W2": W2, "G2": np.ascontiguousarray(G2),
        "W1eK": W1eK, "W2eK": W2eK, "W3m": W3m,
        "w": np.ascontiguousarray(w),
        "kb1": kb1, "kb2": kb2, "b3": b3c,
    }
    in_maps = []
    for c in range(NCORES):
        im = dict(shared)
        im["xT"] = np.ascontiguousarray(xT[:, c * TOK:(c + 1) * TOK])
        in_maps.append(im)
    return in_maps


def kernel(**inputs):
    from concourse.bass_utils import run_bass_kernel_spmd

    if "nc" not in _CACHE:
        _CACHE["nc"] = _build_program()
    nc = _CACHE["nc"]

    in_maps = _prep_host(inputs)
    res = run_bass_kernel_spmd(nc, in_maps, core_ids=list(range(NCORES)))

    v = np.empty((NTOK, F1), np.float32)
    o = np.empty((NTOK, SZ), np.float32)
    for c in range(NCORES):
        rr = res.results[c]
        v[c * TOK:(c + 1) * TOK, :] = rr["vT"].T
        o[c * TOK:(c + 1) * TOK, :] = rr["oT"].T
    o = o.reshape(B, T, ACTION_DIM, ATOMS)
    v = v.reshape(B, T, F1)
    return o, v


# revision 47
# speedup vs baseline: 1.1630x; 1.0016x over previous
"""Trainium2 Bass kernel for nn_ActorMultiEnv_v2 (NALU actor network).

Strategy:
  - Data-parallel: 16384 tokens sharded as 2048 tokens per core across 8 cores.
  - Feature-major on-chip layout: activations stored [feature(partition), token(free)],
    so every matmul uses the weight in natural [in, out] layout as the stationary
    operand and never needs an on-device transpose.
  - All matmuls run as float32r (full fp32 data, reduced-precision multiplier,
    1 cycle/row on the PE -- 4x faster than true fp32).
  - sigmoid(g) is computed as 0.5 + 0.5*tanh(g/2) so the whole network needs only
    two ACT table sets (natural_log_exp and gelu_apprx_tanh); token superchunks are
    staggered through the two table "eras" to keep table swaps ~O(10).
  - Host does weight-only preparation (tanh(W_hat)*sigmoid(M_hat), the block-diagonal
    Keff = kron(I, kn)[:f1,:f1]*we, W3*we, bias folds) and input/output layout
    (transpose to feature-major shards, gather + transpose back).
"""

import sys

import numpy as np

for _p in ("/opt/trn_rl_repo",):
    if _p not in sys.path:
        sys.path.append(_p)

# ---- problem constants (hardcoded per harness contract) ----
B, T, FEAT, F1 = 128, 128, 256, 512
ACTION_DIM, ATOMS = 6, 32
SZ = ACTION_DIM * ATOMS            # 192
NCORES = 8
NTOK = B * T                        # 16384
TOK = NTOK // NCORES                # 2048 tokens per core
SUP = 4                             # superchunks per core
TS = TOK // SUP                     # 512 tokens per superchunk (== one PSUM chunk)
P = 128
M1 = F1 // P                        # 4 output-feature tiles
K1 = FEAT // P                      # 2 input k-tiles (layer 1)
K2 = F1 // P                        # 4 k-tiles (f1-sized contractions)
KE = 2 * F1 // P                    # 8 k-tiles (fc1e)
MO = 2                              # o-matmul M tiles (128 + 64)

EXP_LO = float(np.exp(np.float64(np.float32(-20.0))))
EXP_HI = float(np.exp(np.float64(np.float32(20.0))))

_CACHE = {}


def _build_program():
    from contextlib import ExitStack

    import concourse.tile as tile
    from concourse import bacc, mybir

    f32 = mybir.dt.float32
    f32r = mybir.dt.float32r
    AFT = mybir.ActivationFunctionType
    ALU = mybir.AluOpType

    nc = bacc.Bacc("TRN2", target_bir_lowering=False, debug=False,
                   num_devices=NCORES)

    def din(name, shape, dt=f32):
        return nc.dram_tensor(name, shape, dt, kind="ExternalInput").ap()

    def dout(name, shape):
        return nc.dram_tensor(name, shape, f32, kind="ExternalOutput").ap()

    io = {
        "xT": din("xT", (FEAT, TOK)),
        "W1": din("W1", (FEAT, F1)),
        "G1": din("G1", (FEAT, F1)),
        "W2": din("W2", (F1, F1), f32r),
        "G2": din("G2", (F1, F1), f32r),
        "W1eK": din("W1eK", (2 * F1, F1), f32r),
        "W2eK": din("W2eK", (F1, F1), f32r),
        "W3m": din("W3m", (F1, F1), f32r),
        "w": din("w", (F1, SZ), f32r),
        "kb1": din("kb1", (F1, 1)),
        "kb2": din("kb2", (F1, 1)),
        "b3": din("b3", (F1, 1)),
        "vT": dout("vT", (F1, TOK)),
        "oT": dout("oT", (SZ, TOK)),
    }

    with tile.TileContext(nc) as tc, ExitStack() as ctx:
        wp = ctx.enter_context(tc.tile_pool(name="wp", bufs=1))
        ap_ = ctx.enter_context(tc.tile_pool(name="act", bufs=1))
        tp = ctx.enter_context(tc.tile_pool(name="tmp", bufs=2))

        def wload(key, n_ktiles, width):
            ts = []
            for k in range(n_ktiles):
                t = wp.tile([P, width], f32r, name=f"{key}_{k}")
                nc.sync.dma_start(t[:], io[key][k * P:(k + 1) * P, :])
                ts.append(t)
            return ts

        def load_x(s):
            row = []
            for k in range(K1):
                xf = tp.tile([P, TS], f32, name=f"xf{k}", tag=f"xf{k}", bufs=2)
                nc.sync.dma_start(xf[:], io["xT"][k * P:(k + 1) * P,
                                                  s * TS:(s + 1) * TS])
                row.append(xf)
            return row

        xpre = [load_x(0), load_x(1)]
        def wsplit(key, n_ktiles):
            hs, ls = [], []
            for k in range(n_ktiles):
                wf = tp.tile([P, F1], f32, name=f"{key}f{k}", tag="wpf", bufs=1)
                nc.sync.dma_start(wf[:], io[key][k * P:(k + 1) * P, :])
                wh = wp.tile([P, F1], f32r, name=f"{key}h_{k}")
                nc.vector.tensor_copy(wh[:], wf[:])
                we_ = tp.tile([P, F1], f32, name=f"{key}e{k}", tag="echo", bufs=1)
                nc.vector.tensor_copy(we_[:], wh[:])
                wl = wp.tile([P, F1], f32r, name=f"{key}l_{k}")
                nc.vector.tensor_sub(wl[:], wf[:], we_[:])
                hs.append(wh)
                ls.append(wl)
            return hs, ls

        def bload(key):
            ts = []
            for m in range(M1):
                t = wp.tile([P, 1], f32, name=f"{key}b_{m}")
                nc.sync.dma_start(t[:], io[key][m * P:(m + 1) * P, :])
                ts.append(t)
            return ts

        kb1t = bload("kb1")
        kb2t = bload("kb2")
        b3t = bload("b3")

        epst = wp.tile([P, 1], f32, name="eps_b")
        nc.gpsimd.memset(epst[:], 1e-7)

        def ms(m):
            return slice(m * P, (m + 1) * P)

        st = [dict() for _ in range(SUP)]

        from concourse.tile import add_dep_helper
        _cur_load = [None]
        _era_ops = []

        def act(*args, **kw):
            bi = nc.scalar.activation(*args, **kw)
            ins = getattr(bi, "ins", bi)
            if _cur_load[0] is not None:
                add_dep_helper(ins, _cur_load[0], sync=False,
                               reason="act-era gate")
            _era_ops.append(ins)
            return bi

        from concourse.hw_specs import get_activation_tables
        _table_names = list(get_activation_tables(nc.m.arch).keys())
        _SET_A = _table_names.index("natural_log_exp_and_others")
        _SET_B = _table_names.index("gelu_apprx_tanh_and_others")

        def era_load(set_id):
            inst = mybir.InstLoadActFuncSet(
                name=nc.get_next_instruction_name(), ins=[], outs=[],
                act_func_set_id=set_id)
            bi = nc.scalar.add_instruction(inst)
            ins = getattr(bi, "ins", bi)
            for prev in _era_ops:
                add_dep_helper(ins, prev, sync=False,
                               reason="act-era barrier")
            _era_ops.clear()
            _cur_load[0] = ins

        # ------------------------------------------------------------------
        def emit_prefetch_x(s):
            st[s]["xf"] = xpre[s] if s < len(xpre) else load_x(s)

        def emit_lnx(s):
            """split x into xr/xl, abs+ln on fp32 x, split lx into lxr/lxl."""
            S = st[s]
            xparts = []
            for k in range(K1):
                xf = S["xf"][k]
                xr = ap_.tile([P, TS], f32r, name=f"xr{k}", tag=f"xr{k}", bufs=1)
                nc.vector.tensor_copy(xr[:], xf[:])
                xe = tp.tile([P, TS], f32, name=f"xec{k}", tag="echo", bufs=1)
                nc.vector.tensor_copy(xe[:], xr[:])
                xl = ap_.tile([P, TS], f32r, name=f"xl{k}", tag=f"xl{k}", bufs=1)
                nc.vector.tensor_sub(xl[:], xf[:], xe[:])
                xparts.append((xf, xr, xl))
            S["xparts"] = xparts
            lxrk, lxlk = [], []
            for k in range(K1):
                xf = S["xparts"][k][0]
                lxf = tp.tile([P, TS], f32, name=f"lxf{k}", tag=f"lxf{k}", bufs=1)
                act(lxf[:], xf[:], AFT.Abs)
                act(lxf[:], lxf[:], AFT.Ln, bias=epst[:])
                lxr = ap_.tile([P, TS], f32r, name=f"lxr{k}", tag=f"lxr{k}",
                               bufs=1)
                nc.vector.tensor_copy(lxr[:], lxf[:])
                le = tp.tile([P, TS], f32, name=f"le{k}", tag="echo", bufs=1)
                nc.vector.tensor_copy(le[:], lxr[:])
                lxl = ap_.tile([P, TS], f32r, name=f"lxl{k}", tag=f"lxl{k}",
                               bufs=1)
                nc.vector.tensor_sub(lxl[:], lxf[:], le[:])
                lxrk.append(lxr)
                lxlk.append(lxl)
            S["lxrk"], S["lxlk"] = lxrk, lxlk

        def l1_terms(hs, ls, parts_r, parts_l):
            """k-sequence for a 3-term split matmul group (6 k-tiles)."""
            seq = []
            for k in range(K1):
                seq.append((hs[k], parts_r[k]))
            for k in range(K1):
                seq.append((hs[k], parts_l[k]))
            for k in range(K1):
                seq.append((ls[k], parts_r[k]))
            return seq

        def era_A_L1(s):
            """ln/exp-table era: a1/p1 split matmuls, exp."""
            S = st[s]
            xr = [pk[1] for pk in S["xparts"]]
            xl = [pk[2] for pk in S["xparts"]]
            with tc.tile_pool(name=f"psA1_{s}", bufs=1, space="PSUM") as pp:
                a1sb, m1c = [], []
                for m in range(M1):
                    pa = pp.tile([P, TS], f32, name=f"a1_{m}", tag="a1", bufs=2)
                    seq = l1_terms(W1h, W1l, xr, xl)
                    for j, (wt_, xt_) in enumerate(seq):
                        nc.tensor.matmul(pa[:], wt_[:, ms(m)], xt_[:],
                                         start=(j == 0), stop=(j == len(seq) - 1))
                    a1 = ap_.tile([P, TS], f32, name=f"a1sb{m}", tag=f"a1sb{m}",
                                  bufs=1)
                    nc.vector.tensor_copy(a1[:], pa[:])
                    a1sb.append(a1)

                    pm = pp.tile([P, TS], f32, name=f"p1_{m}", tag="p1", bufs=2)
                    seq = l1_terms(W1h, W1l, S["lxrk"], S["lxlk"])
                    for j, (wt_, xt_) in enumerate(seq):
                        nc.tensor.matmul(pm[:], wt_[:, ms(m)], xt_[:],
                                         start=(j == 0), stop=(j == len(seq) - 1))
                    mc = ap_.tile([P, TS], f32, name=f"m1c{m}", tag=f"m1c{m}",
                                  bufs=1)
                    act(mc[:], pm[:], AFT.Exp)
                    m1c.append(mc)
                S["a1sb"], S["m1c"] = a1sb, m1c

        # ------------------------------------------------------------------
        def era_B_L1(s):
            """gelu-table era: g1 matmul + tanh gate, combine, h11/h12."""
            S = st[s]
            with tc.tile_pool(name=f"psB1_{s}", bufs=1, space="PSUM") as pp:
                a1sb, m1c = S["a1sb"], S["m1c"]
                # independent ACT work first: h11 gelus, then gate tanhs
                h11 = []
                for m in range(M1):
                    h1 = ap_.tile([P, TS], f32r, name=f"h11_{m}", tag=f"h11_{m}",
                                  bufs=1)
                    act(h1[:], a1sb[m][:], AFT.Gelu_apprx_tanh)
                    h11.append(h1)
                t1s, ds = [], []
                xr = [pk[1] for pk in S["xparts"]]
                xl = [pk[2] for pk in S["xparts"]]
                for m in range(M1):
                    pg = pp.tile([P, TS], f32, name=f"g1_{m}", tag="g1", bufs=2)
                    seq = l1_terms(G1h, G1l, xr, xl)
                    for j, (wt_, xt_) in enumerate(seq):
                        nc.tensor.matmul(pg[:], wt_[:, ms(m)], xt_[:],
                                         start=(j == 0), stop=(j == len(seq) - 1))
                    t1 = tp.tile([P, TS], f32, name=f"t1_{m}", tag=f"tg{m}", bufs=1)
                    act(t1[:], pg[:], AFT.Tanh, scale=0.5)
                    t1s.append(t1)
                    d = tp.tile([P, TS], f32, name=f"d1_{m}", tag="d", bufs=4)
                    nc.vector.tensor_sub(d[:], a1sb[m][:], m1c[m][:])
                    ds.append(d)
                # h12 feeds ln(|.|) downstream, so its gelu must be exact
                # (LUT error near zero explodes through the log). Compute
                # z = gated combine, then qp = z*(1+tanh(c*(z+0.044715 z^3)))
                # = 2*gelu(z) via tanh LUT + DVE; the 0.5 folds into the Ln
                # scale in era_A_L2.
                C1 = float(np.sqrt(2.0 / np.pi))
                C2 = float(np.sqrt(2.0 / np.pi) * 0.044715)
                h12 = []
                for m in range(M1):
                    nc.vector.tensor_mul(t1s[m][:], t1s[m][:], ds[m][:])
                    nc.vector.tensor_add(ds[m][:], ds[m][:], t1s[m][:])
                    nc.vector.scalar_tensor_tensor(ds[m][:], ds[m][:], 0.5,
                                                   m1c[m][:], ALU.mult, ALU.add)
                    z = ds[m]
                    s1 = t1s[m]
                    nc.vector.tensor_mul(s1[:], z[:], z[:])
                    nc.vector.tensor_scalar(s1[:], s1[:], C2, C1, ALU.mult,
                                            ALU.add)
                    nc.vector.tensor_mul(s1[:], s1[:], z[:])
                    act(s1[:], s1[:], AFT.Tanh)
                    h2 = ap_.tile([P, TS], f32, name=f"h12_{m}", tag=f"h12_{m}",
                                  bufs=1)
                    nc.vector.scalar_tensor_tensor(h2[:], s1[:], 1.0, z[:],
                                                   ALU.add, ALU.mult)
                    h12.append(h2)
                S["h11"], S["h12"] = h11, h12

        # ------------------------------------------------------------------
        def era_A_L2(s):
            """ln/exp era for layer-2: lx2, p2/a2/g2 matmuls, exp, clip, evacs."""
            S = st[s]
            h11, h12 = S["h11"], S["h12"]
            with tc.tile_pool(name=f"psA2_{s}", bufs=1, space="PSUM") as pp:
                # era-agnostic PE work first so its DVE evacs are ready for era B
                a2sb, g2sb = [], []
                for m in range(M1):
                    pa = pp.tile([P, TS], f32, name=f"a2_{m}", tag="a2", bufs=1)
                    for k in range(K2):
                        nc.tensor.matmul(pa[:], W2t[k][:, ms(m)], h11[k][:],
                                         start=(k == 0), stop=(k == K2 - 1))
                    a2 = ap_.tile([P, TS], f32r, name=f"a2sb{m}", tag=f"a2sb{m}",
                                  bufs=1)
                    nc.vector.tensor_copy(a2[:], pa[:])
                    a2sb.append(a2)

                    pg = pp.tile([P, TS], f32, name=f"g2_{m}", tag="g2", bufs=1)
                    for k in range(K2):
                        nc.tensor.matmul(pg[:], G2t[k][:, ms(m)], h11[k][:],
                                         start=(k == 0), stop=(k == K2 - 1))
                    gv = ap_.tile([P, TS], f32, name=f"g2sb{m}", tag=f"g2sb{m}",
                                  bufs=1)
                    nc.vector.tensor_copy(gv[:], pg[:])
                    g2sb.append(gv)
                lx2 = []
                for k in range(K2):
                    l2 = ap_.tile([P, TS], f32r, name=f"lx2_{k}", tag=f"lx2_{k}",
                                  bufs=1)
                    act(l2[:], h12[k][:], AFT.Abs)
                    act(l2[:], l2[:], AFT.Ln, bias=epst[:], scale=0.5)
                    lx2.append(l2)
                m2c = []
                for m in range(M1):
                    pm = pp.tile([P, TS], f32, name=f"p2_{m}", tag="p2", bufs=2)
                    for k in range(K2):
                        nc.tensor.matmul(pm[:], W2t[k][:, ms(m)], lx2[k][:],
                                         start=(k == 0), stop=(k == K2 - 1))
                    mc = ap_.tile([P, TS], f32r, name=f"m2c{m}", tag=f"m2c{m}",
                                  bufs=1)
                    act(mc[:], pm[:], AFT.Exp)
                    m2c.append(mc)
                S["m2c"], S["a2sb"], S["g2sb"] = m2c, a2sb, g2sb

        # ------------------------------------------------------------------
        def era_B_L2comb(s):
            """gelu era: layer-2 combine; ha/ht overwrite a2sb/m2c in place."""
            S = st[s]
            a2sb, m2c, g2sb = S["a2sb"], S["m2c"], S["g2sb"]
            t2s, ds = [], []
            for m in range(M1):
                t2 = tp.tile([P, TS], f32, name=f"t2_{m}", tag=f"tg{m}", bufs=1)
                act(t2[:], g2sb[m][:], AFT.Tanh, scale=0.5)
                t2s.append(t2)
                d = tp.tile([P, TS], f32, name=f"d2_{m}", tag="d", bufs=4)
                nc.vector.tensor_sub(d[:], a2sb[m][:].bitcast(f32),
                                     m2c[m][:].bitcast(f32))
                ds.append(d)
            ha = []
            for m in range(M1):
                # gelu(a2) written over the (f32r) a2sb tile in place
                act(a2sb[m][:], a2sb[m][:].bitcast(f32), AFT.Gelu_apprx_tanh)
                ha.append(a2sb[m])
            ht = []
            for m in range(M1):
                nc.vector.tensor_mul(t2s[m][:], t2s[m][:], ds[m][:])
                nc.vector.tensor_add(ds[m][:], ds[m][:], t2s[m][:])
                nc.vector.scalar_tensor_tensor(ds[m][:], ds[m][:], 0.5,
                                               m2c[m][:].bitcast(f32),
                                               ALU.mult, ALU.add)
                act(m2c[m][:], ds[m][:], AFT.Gelu_apprx_tanh)
                ht.append(m2c[m])
            S["ha"], S["ht"] = ha, ht

        def era_fc(s):
            """gelu era: folded fc chain, v and o out (runs one period late)."""
            S = st[s]
            tsl = slice(s * TS, (s + 1) * TS)
            ha, ht = S["ha"], S["ht"]
            with tc.tile_pool(name=f"psFC_{s}", bufs=1, space="PSUM") as pp:
                # fc1e+Keff folded: xa1 = gelu([ha;ht] @ (W1e@Keff) + b1e@Keff)
                xa1 = []
                for m in range(M1):
                    pz = pp.tile([P, TS], f32, name=f"z1_{m}", tag="z1", bufs=2)
                    for k in range(KE):
                        srck = ha[k] if k < K2 else ht[k - K2]
                        nc.tensor.matmul(pz[:], W1eKt[k][:, ms(m)], srck[:],
                                         start=(k == 0), stop=(k == KE - 1))
                    x1 = ap_.tile([P, TS], f32r, name=f"xa1_{m}", tag=f"xe_{m}",
                                  bufs=2)
                    act(x1[:], pz[:], AFT.Gelu_apprx_tanh,
                                         bias=kb1t[m][:])
                    xa1.append(x1)
                # o = xa1 @ w
                for mo in range(MO):
                    mw = P if mo == 0 else SZ - P
                    po = pp.tile([mw, TS], f32, name=f"zo_{mo}", tag="zx", bufs=2)
                    for k in range(K2):
                        nc.tensor.matmul(po[:], wt[k][:, mo * P:mo * P + mw],
                                         xa1[k][:],
                                         start=(k == 0), stop=(k == K2 - 1))
                    oo = tp.tile([mw, TS], f32, name=f"oo_{mo}", tag="outb", bufs=2)
                    act(oo[:], po[:], AFT.Copy)
                    nc.sync.dma_start(io["oT"][mo * P:mo * P + mw, tsl], oo[:])
                # fc2e+Keff folded: xa2 = gelu(xa1 @ (W2e@Keff) + b2e@Keff)
                xa2 = []
                for m in range(M1):
                    pz = pp.tile([P, TS], f32, name=f"z2_{m}", tag="zx", bufs=2)
                    for k in range(K2):
                        nc.tensor.matmul(pz[:], W2eKt[k][:, ms(m)], xa1[k][:],
                                         start=(k == 0), stop=(k == K2 - 1))
                    x2 = ap_.tile([P, TS], f32r, name=f"xa2_{m}", tag=f"xe_{m}",
                                  bufs=2)
                    act(x2[:], pz[:], AFT.Gelu_apprx_tanh,
                                         bias=kb2t[m][:])
                    xa2.append(x2)
                # fc3: v = xa2 @ W3m + b3
                for m in range(M1):
                    pz = pp.tile([P, TS], f32, name=f"zv_{m}", tag="zx", bufs=2)
                    for k in range(K2):
                        nc.tensor.matmul(pz[:], W3mt[k][:, ms(m)], xa2[k][:],
                                         start=(k == 0), stop=(k == K2 - 1))
                    vo = tp.tile([P, TS], f32, name=f"vo_{m}", tag="outb", bufs=2)
                    act(vo[:], pz[:], AFT.Identity, bias=b3t[m][:])
                    nc.sync.dma_start(io["vT"][m * P:(m + 1) * P, tsl], vo[:])

        # ---- staggered era schedule: L1(s) | L2(s-1) ----
        era_load(_SET_A)
        emit_prefetch_x(0)
        emit_prefetch_x(1)
        emit_lnx(0)
        W1h, W1l = wsplit("W1", K1)
        G1h, G1l = wsplit("G1", K1)
        W2t = wload("W2", K2, F1)
        G2t = wload("G2", K2, F1)
        W1eKt = wload("W1eK", KE, F1)
        W2eKt = wload("W2eK", K2, F1)
        W3mt = wload("W3m", K2, F1)
        wt = wload("w", K2, SZ)
        era_A_L1(0)
        era_load(_SET_B)
        era_B_L1(0)
        for s in range(1, SUP):
            era_load(_SET_A)
            if s + 1 < SUP:
                emit_prefetch_x(s + 1)
            emit_lnx(s)
            era_A_L2(s - 1)
            era_A_L1(s)
            era_load(_SET_B)
            era_B_L2comb(s - 1)
            era_B_L1(s)
            era_fc(s - 1)
        era_load(_SET_A)
        era_A_L2(SUP - 1)
        era_load(_SET_B)
        era_B_L2comb(SUP - 1)
        era_fc(SUP - 1)

    nc.compile()
    return nc


def _prep_host(inputs):
    f = np.asarray(inputs["features"], np.float32).reshape(NTOK, FEAT)
    W_hat1 = np.asarray(inputs["W_hat1"], np.float32)
    M_hat1 = np.asarray(inputs["M_hat1"], np.float32)
    G1 = np.asarray(inputs["G1"], np.float32)
    W_hat2 = np.asarray(inputs["W_hat2"], np.float32)
    M_hat2 = np.asarray(inputs["M_hat2"], np.float32)
    G2 = np.asarray(inputs["G2"], np.float32)
    W1e = np.asarray(inputs["W1e"], np.float32)
    b1e = np.asarray(inputs["b1e"], np.float32)
    W2e = np.asarray(inputs["W2e"], np.float32)
    b2e = np.asarray(inputs["b2e"], np.float32)
    W3 = np.asarray(inputs["W3"], np.float32)
    b3 = np.asarray(inputs["b3"], np.float32)
    kn = np.asarray(inputs["kn"], np.float32)
    w = np.asarray(inputs["w"], np.float32)
    we = np.asarray(inputs["we"], np.float32)
    sz_mini = int(inputs["sz_mini"])

    # weight-only preparation (static per model)
    W1 = (np.tanh(W_hat1) * (1.0 / (1.0 + np.exp(-M_hat1)))).astype(np.float32)
    W2 = (np.tanh(W_hat2) * (1.0 / (1.0 + np.exp(-M_hat2)))).astype(np.float32)
    Keff = (np.kron(np.eye(sz_mini, dtype=np.float32), kn)[:F1, :F1] * we)
    Keff64 = Keff.astype(np.float64)
    # fold the (linear) Keff matmul into the preceding dense weights:
    # (h @ W1e + b1e) @ Keff == h @ (W1e@Keff) + (b1e@Keff)
    W1eK = np.ascontiguousarray((W1e.astype(np.float64) @ Keff64).astype(np.float32))
    W2eK = np.ascontiguousarray((W2e.astype(np.float64) @ Keff64).astype(np.float32))
    W3m = np.ascontiguousarray(W3 * we, np.float32)
    kb1 = (b1e.astype(np.float64) @ Keff64).astype(np.float32).reshape(F1, 1)
    kb2 = (b2e.astype(np.float64) @ Keff64).astype(np.float32).reshape(F1, 1)
    b3c = b3.astype(np.float32).reshape(F1, 1)

    xT = np.ascontiguousarray(f.T)  # [FEAT, NTOK]

    shared = {
        "W1": W1, "G1": np.ascontiguousarray(G1),
        "W2": W2, "G2": np.ascontiguousarray(G2),
        "W1eK": W1eK, "W2eK": W2eK, "W3m": W3m,
        "w": np.ascontiguousarray(w),
        "kb1": kb1, "kb2": kb2, "b3": b3c,
    }
    in_maps = []
    for c in range(NCORES):
        im = dict(shared)
        im["xT"] = np.ascontiguousarray(xT[:, c * TOK:(c + 1) * TOK])
        in_maps.append(im)
    return in_maps


def kernel(**inputs):
    from concourse.bass_utils import run_bass_kernel_spmd

    if "nc" not in _CACHE:
        _CACHE["nc"] = _build_program()
    nc = _CACHE["nc"]

    in_maps = _prep_host(inputs)
    res = run_bass_kernel_spmd(nc, in_maps, core_ids=list(range(NCORES)))

    v = np.empty((NTOK, F1), np.float32)
    o = np.empty((NTOK, SZ), np.float32)
    for c in range(NCORES):
        rr = res.results[c]
        v[c * TOK:(c + 1) * TOK, :] = rr["vT"].T
        o[c * TOK:(c + 1) * TOK, :] = rr["oT"].T
    o = o.reshape(B, T, ACTION_DIM, ATOMS)
    v = v.reshape(B, T, F1)
    return o, v
